# revision 2
# baseline (speedup 1.0000x reference)
"""Trainium2 Bass kernel for nn_CentersDistance.

logits[k, n] = -||centers[k] - inputs[n]||^2
             = 2*(centers @ inputs.T)[k, n] - ||centers[k]||^2 - ||inputs[n]||^2

Strategy (8 NeuronCores, data-parallel over N):
  * host: transpose both operands so the contraction dim D lands on the SBUF
    partition axis ([D, K] and [D, N] layouts), fold the factor 2 into the
    inputs, precompute the (exact, float64) norm terms.
  * each core: 1024x1024x1024 matmul in bf16 (fp32 PSUM accumulation),
    epilogue on DVE adds -||c||^2 (per-partition scalar) and -||x||^2
    (broadcast row) in a single scalar_tensor_tensor op, store fp32.
"""

import threading

import numpy as np
import ml_dtypes

import concourse.bass as bass
import concourse.mybir as mybir
import concourse.tile as tile
from concourse import bacc
from concourse.bass_utils import run_bass_kernel_spmd

N_CORES = 8
N, K, D = 8192, 1024, 1024
NSH = N // N_CORES  # per-core slab of inputs
P = 128             # SBUF partitions
NF = 512            # matmul moving free dim (one fp32 PSUM bank)

D_TILES = D // P    # 8
M_TILES = K // P    # 8
H_TILES = NSH // NF # 2

_DT = mybir.dt.bfloat16
_NP_DT = ml_dtypes.bfloat16

_cache = threading.local()


def _build_nc():
    nc = bacc.Bacc(
        "TRN2", target_bir_lowering=False, debug=False, num_devices=N_CORES
    )
    ct = nc.dram_tensor("ct", [D, K], _DT, kind="ExternalInput").ap()
    xt = nc.dram_tensor("xt", [D, NSH], _DT, kind="ExternalInput").ap()
    ncsq = nc.dram_tensor(
        "ncsq", [P, M_TILES], mybir.dt.float32, kind="ExternalInput"
    ).ap()
    nxsq = nc.dram_tensor(
        "nxsq", [P, NSH], mybir.dt.float32, kind="ExternalInput"
    ).ap()
    out = nc.dram_tensor("out", [K, NSH], mybir.dt.float32, kind="ExternalOutput").ap()

    ct_r = ct.rearrange("(t p) k -> t p k", p=P)
    xt_r = xt.rearrange("(t p) n -> t p n", p=P)
    out_r = out.rearrange("(m p) n -> m p n", p=P)

    with tile.TileContext(nc) as tc:
        with (
            tc.tile_pool(name="w", bufs=1) as wpool,
            tc.tile_pool(name="c", bufs=1) as cpool,
            tc.tile_pool(name="o", bufs=4) as opool,
            tc.tile_pool(name="ps", bufs=8, space="PSUM") as pspool,
        ):
            ncsq_sb = cpool.tile([P, M_TILES], mybir.dt.float32, tag="ncsq")
            nc.sync.dma_start(ncsq_sb[:], ncsq)
            nxsq_sb = cpool.tile([P, NSH], mybir.dt.float32, tag="nxsq")
            nc.sync.dma_start(nxsq_sb[:], nxsq)

            ct_sb = []
            xt_sb = []
            for d in range(D_TILES):
                t = wpool.tile([P, K], _DT, tag=f"ct{d}")
                nc.sync.dma_start(t[:], ct_r[d])
                ct_sb.append(t)
                t = wpool.tile([P, NSH], _DT, tag=f"xt{d}")
                nc.sync.dma_start(t[:], xt_r[d])
                xt_sb.append(t)

            # Two passes of 4 m-tiles x 2 h-tiles = 8 concurrent PSUM banks.
            # d is outermost within a pass so the first pass paces with the
            # streaming ct/xt DMAs; the second pass runs from resident SBUF.
            for half in range(2):
                ms = range(half * M_TILES // 2, (half + 1) * M_TILES // 2)
                ps = {}
                for m in ms:
                    for h in range(H_TILES):
                        ps[(m, h)] = pspool.tile([P, NF], mybir.dt.float32, tag="ps", name=f"ps_{m}_{h}")
                for d in range(D_TILES):
                    for m in ms:
                        for h in range(H_TILES):
                            nc.tensor.matmul(
                                ps[(m, h)][:],
                                ct_sb[d][:, m * P : (m + 1) * P],
                                xt_sb[d][:, h * NF : (h + 1) * NF],
                                start=(d == 0),
                                stop=(d == D_TILES - 1),
                            )
                for m in ms:
                    for h in range(H_TILES):
                        ot = opool.tile([P, NF], mybir.dt.float32, tag="ot")
                        nc.vector.scalar_tensor_tensor(
                            ot[:],
                            ps[(m, h)][:],
                            ncsq_sb[:, m : m + 1],
                            nxsq_sb[:, h * NF : (h + 1) * NF],
                            op0=mybir.AluOpType.add,
                            op1=mybir.AluOpType.add,
                        )
                        nc.sync.dma_start(out_r[m][:, h * NF : (h + 1) * NF], ot[:])

    nc.compile()
    return nc


def _get_nc():
    if not hasattr(_cache, "nc"):
        _cache.nc = _build_nc()
    return _cache.nc


def kernel(inputs, centers, _trace=False):
    inputs = np.asarray(inputs, dtype=np.float32)
    centers = np.asarray(centers, dtype=np.float32)

    csq = np.sum(centers.astype(np.float64) ** 2, axis=1)
    xsq = np.sum(inputs.astype(np.float64) ** 2, axis=1)

    ct = np.ascontiguousarray(centers.T).astype(_NP_DT)
    xt2 = np.ascontiguousarray((2.0 * inputs).T.astype(_NP_DT))
    ncsq = np.ascontiguousarray(
        (-csq).reshape(M_TILES, P).T.astype(np.float32)
    )

    in_maps = []
    for i in range(N_CORES):
        sl = slice(i * NSH, (i + 1) * NSH)
        in_maps.append(
            {
                "ct": ct,
                "xt": np.ascontiguousarray(xt2[:, sl]),
                "ncsq": ncsq,
                "nxsq": np.ascontiguousarray(
                    np.broadcast_to(-xsq[sl], (P, NSH))
                ).astype(np.float32),
            }
        )

    nc = _get_nc()
    res = run_bass_kernel_spmd(
        nc, in_maps, core_ids=list(range(N_CORES)), trace=_trace
    )
    if _trace:
        kernel.last_results = res
    out = np.concatenate([r["out"] for r in res.results], axis=1)
    return out


# revision 3
# speedup vs baseline: 1.1024x; 1.1024x over previous
"""Trainium2 Bass kernel for nn_CentersDistance.

logits[k, n] = -||centers[k] - inputs[n]||^2
             = 2*(centers @ inputs.T)[k, n] - ||centers[k]||^2 - ||inputs[n]||^2

Strategy (8 NeuronCores, data-parallel over N):
  * host: transpose both operands so the contraction dim D lands on the SBUF
    partition axis ([D, K] and [D, N] layouts), fold the factor 2 into the
    inputs, precompute the (exact, float64) norm terms.
  * each core: 1024x1024x1024 matmul in bf16 (fp32 PSUM accumulation),
    epilogue on DVE adds -||c||^2 (per-partition scalar) and -||x||^2
    (broadcast row) in a single scalar_tensor_tensor op, store fp32.
"""

import threading

import numpy as np
import ml_dtypes

import concourse.bass as bass
import concourse.mybir as mybir
import concourse.tile as tile
from concourse import bacc
from concourse.bass_utils import run_bass_kernel_spmd

N_CORES = 8
N, K, D = 8192, 1024, 1024
NSH = N // N_CORES  # per-core slab of inputs
P = 128             # SBUF partitions
NF = 512            # matmul moving free dim (one fp32 PSUM bank)

D_TILES = D // P    # 8
M_TILES = K // P    # 8
H_TILES = NSH // NF # 2

_DT = mybir.dt.bfloat16
_NP_DT = ml_dtypes.bfloat16

_cache = threading.local()


def _build_nc():
    nc = bacc.Bacc(
        "TRN2", target_bir_lowering=False, debug=False, num_devices=N_CORES
    )
    ct = nc.dram_tensor("ct", [D, K], _DT, kind="ExternalInput").ap()
    xt = nc.dram_tensor("xt", [D, NSH], _DT, kind="ExternalInput").ap()
    ncsq = nc.dram_tensor(
        "ncsq", [P, M_TILES], mybir.dt.float32, kind="ExternalInput"
    ).ap()
    nxsq = nc.dram_tensor(
        "nxsq", [P, NSH], mybir.dt.float32, kind="ExternalInput"
    ).ap()
    out = nc.dram_tensor("out", [K, NSH], mybir.dt.float32, kind="ExternalOutput").ap()

    ct_r = ct.rearrange("(t p) k -> t p k", p=P)
    xt_r = xt.rearrange("(t p) n -> t p n", p=P)
    out_r = out.rearrange("(m p) n -> m p n", p=P)

    with tile.TileContext(nc) as tc:
        with (
            tc.tile_pool(name="w", bufs=1) as wpool,
            tc.tile_pool(name="c", bufs=1) as cpool,
            tc.tile_pool(name="o", bufs=4) as opool,
            tc.tile_pool(name="ps", bufs=7, space="PSUM") as pspool,
            tc.tile_pool(name="wu", bufs=1, space="PSUM") as wupool,
        ):
            # PE warm-up: ~dummy matmuls on a zeroed tile, no data deps, so
            # the tensor engine is busy during the load phase and the HAM
            # clock gate is fully open (2.4 GHz) when the real matmuls start.
            wu_sb = cpool.tile([P, NF], _DT, tag="wu_sb")
            nc.gpsimd.memset(wu_sb[:], 0.0)
            wu_ps = wupool.tile([P, NF], mybir.dt.float32, tag="wu_ps")
            for _ in range(24):
                nc.tensor.matmul(
                    wu_ps[:], wu_sb[:, 0:P], wu_sb[:], start=True, stop=True
                )

            ct_sb = []
            xt_sb = []
            for d in range(D_TILES):
                t = wpool.tile([P, K], _DT, tag=f"ct{d}")
                nc.sync.dma_start(t[:], ct_r[d])
                ct_sb.append(t)
                t = wpool.tile([P, NSH], _DT, tag=f"xt{d}")
                nc.sync.dma_start(t[:], xt_r[d])
                xt_sb.append(t)
                if d == 1:
                    # epilogue constants — needed much later than the first
                    # ct/xt tiles, so don't put them ahead in the DMA queue
                    ncsq_sb = cpool.tile([P, M_TILES], mybir.dt.float32, tag="ncsq")
                    nc.sync.dma_start(ncsq_sb[:], ncsq)
                    nxsq_sb = cpool.tile([P, NSH], mybir.dt.float32, tag="nxsq")
                    nc.sync.dma_start(nxsq_sb[:], nxsq)

            def epilogue(m, h, ps):
                ot = opool.tile([P, NF], mybir.dt.float32, tag="ot", name=f"ot{m}_{h}")
                nc.vector.scalar_tensor_tensor(
                    ot[:],
                    ps[:],
                    ncsq_sb[:, m : m + 1],
                    nxsq_sb[:, h * NF : (h + 1) * NF],
                    op0=mybir.AluOpType.add,
                    op1=mybir.AluOpType.add,
                )
                nc.sync.dma_start(out_r[m][:, h * NF : (h + 1) * NF], ot[:])

            # Pass 1 (m-tiles 0-3): d outermost so the matmuls pace with the
            # streaming ct/xt DMAs; 8 PSUM banks accumulate concurrently.
            ms = range(M_TILES // 2)
            ps = {}
            for m in ms:
                for h in range(H_TILES):
                    ps[(m, h)] = pspool.tile(
                        [P, NF], mybir.dt.float32, tag="ps", name=f"ps_{m}_{h}"
                    )
            for d in range(D_TILES):
                for m in ms:
                    for h in range(H_TILES):
                        nc.tensor.matmul(
                            ps[(m, h)][:],
                            ct_sb[d][:, m * P : (m + 1) * P],
                            xt_sb[d][:, h * NF : (h + 1) * NF],
                            start=(d == 0),
                            stop=(d == D_TILES - 1),
                        )
            for m in ms:
                for h in range(H_TILES):
                    epilogue(m, h, ps[(m, h)])

            # Pass 2 (m-tiles 4-7): everything is resident now, so run d
            # innermost — each (m, h) output retires early and its DVE
            # epilogue + store overlap the remaining matmuls instead of
            # serializing at the kernel tail.
            for m in range(M_TILES // 2, M_TILES):
                for h in range(H_TILES):
                    p2 = pspool.tile(
                        [P, NF], mybir.dt.float32, tag="ps", name=f"ps2_{m}_{h}"
                    )
                    for d in range(D_TILES):
                        nc.tensor.matmul(
                            p2[:],
                            ct_sb[d][:, m * P : (m + 1) * P],
                            xt_sb[d][:, h * NF : (h + 1) * NF],
                            start=(d == 0),
                            stop=(d == D_TILES - 1),
                        )
                    epilogue(m, h, p2)

    nc.compile()
    return nc


def _get_nc():
    if not hasattr(_cache, "nc"):
        _cache.nc = _build_nc()
    return _cache.nc


def kernel(inputs, centers, _trace=False):
    inputs = np.asarray(inputs, dtype=np.float32)
    centers = np.asarray(centers, dtype=np.float32)

    csq = np.sum(centers.astype(np.float64) ** 2, axis=1)
    xsq = np.sum(inputs.astype(np.float64) ** 2, axis=1)

    ct = np.ascontiguousarray(centers.T).astype(_NP_DT)
    xt2 = np.ascontiguousarray((2.0 * inputs).T.astype(_NP_DT))
    ncsq = np.ascontiguousarray(
        (-csq).reshape(M_TILES, P).T.astype(np.float32)
    )

    in_maps = []
    for i in range(N_CORES):
        sl = slice(i * NSH, (i + 1) * NSH)
        in_maps.append(
            {
                "ct": ct,
                "xt": np.ascontiguousarray(xt2[:, sl]),
                "ncsq": ncsq,
                "nxsq": np.ascontiguousarray(
                    np.broadcast_to(-xsq[sl], (P, NSH))
                ).astype(np.float32),
            }
        )

    nc = _get_nc()
    res = run_bass_kernel_spmd(
        nc, in_maps, core_ids=list(range(N_CORES)), trace=_trace
    )
    if _trace:
        kernel.last_results = res
    out = np.concatenate([r["out"] for r in res.results], axis=1)
    return out
